# revision 3
# baseline (speedup 1.0000x reference)
"""ConvLoRA fused kernel for Trainium2 (8 NeuronCores, data-parallel over batch).

Math: conv is linear in its weight, so
    org + outA + outB = conv(x[b], conv_w + wA[b] + wB[b]) + conv_b
One fused per-sample 3x3 conv (256->256ch) in bf16. Per-sample weights are
generated on-device: since both LoRA branches share lora_B,
    wA + wB = B @ (C1@A1 + C2@A2)
so the two branches accumulate into one PSUM "AS" [16, 768] which is then
expanded tap-by-tap via strided matmuls against a zero-padded copy (ASE) --
out-of-segment reads land in zeros, handling the p%3 block boundaries exactly.

Conv streams contiguous 3-row windows (N=388) from a whole-image SBUF buffer
with 130-elem row pitch (halo cols are zero); the 2 junk columns per row
boundary are discarded at PSUM evacuation. One matmul per stationary weight;
bf16 fast-weight-load hides under the 388-cycle matmul.
"""
import sys
sys.path.insert(0, '/opt/trn_rl_repo')
import numpy as np
import ml_dtypes

import concourse.bacc as bacc
import concourse.mybir as mybir
import concourse.tile as tile
from concourse.bass_utils import run_bass_kernel_spmd

f32 = mybir.dt.float32
f32r = mybir.dt.float32r
bf16 = mybir.dt.bfloat16
AF = mybir.ActivationFunctionType

B, CIN, COUT, KS, H, W, R = 16, 256, 256, 3, 128, 128, 16
NCORES = 8
NB = B // NCORES   # 2 samples per core
PITCH = W + 2      # 130: one zero col each side
NTILE = 43         # ceil(128/3): 42 full 3-row tiles + one 2-row tile


def _build_nc():
    nc = bacc.Bacc("TRN2", target_bir_lowering=False, debug=False, num_devices=NCORES)

    x_loc = nc.dram_tensor("x_loc", [NB, CIN, H, W], bf16, kind="ExternalInput")
    wm = nc.dram_tensor("wm", [33, 2 * NB], f32, kind="ExternalInput")
    ew1 = nc.dram_tensor("ew1", [33, 256], f32, kind="ExternalInput")
    w2t = nc.dram_tensor("w2t", [128, 512], f32, kind="ExternalInput")
    b2x = nc.dram_tensor("b2x", [16, 2 * 16 * NB], f32, kind="ExternalInput")
    loraA = nc.dram_tensor("loraA", [32, 768], f32, kind="ExternalInput")
    lba = nc.dram_tensor("lba", [16, 3, 256], f32r, kind="ExternalInput")
    wbase = nc.dram_tensor("wbase", [128, 9, 2, 256], bf16, kind="ExternalInput")
    convb = nc.dram_tensor("convb", [128, 2], f32, kind="ExternalInput")
    out = nc.dram_tensor("out", [NB, COUT, H, W], f32, kind="ExternalOutput")

    with tile.TileContext(nc) as tc:
        from contextlib import ExitStack
        with ExitStack() as ctx:
            cpools = ctx.enter_context(tc.tile_pool(name="consts", bufs=1))
            w18pool = ctx.enter_context(tc.tile_pool(name="w18", bufs=9 * 2 * NB))
            sb_wg = ctx.enter_context(tc.tile_pool(name="sbwg", bufs=1))
            ps_w = ctx.enter_context(tc.tile_pool(name="psw", bufs=1, space="PSUM"))
            xpool = ctx.enter_context(tc.tile_pool(name="ximg", bufs=2 * NB))
            stg = ctx.enter_context(tc.tile_pool(name="stg", bufs=4))

            # zero-padded AS expansion buffers (per sample), zeroed off the
            # critical path on gpsimd
            ASE = []
            for bi in range(NB):
                a = sb_wg.tile([16, 2304], f32r, tag="ase", bufs=NB)
                nc.gpsimd.memset(a[:].bitcast(f32), 0.0)
                ASE.append(a)

            # ---- constants (small / MLP-critical first) ----
            wm_sb = cpools.tile([33, 2 * NB], f32)
            nc.sync.dma_start(wm_sb[:], wm[:])
            ew1_sb = cpools.tile([33, 256], f32)
            nc.sync.dma_start(ew1_sb[:], ew1[:])
            w2t_sb = cpools.tile([128, 512], f32)
            nc.sync.dma_start(w2t_sb[:], w2t[:])
            b2x_sb = cpools.tile([16, 2 * 16 * NB], f32)
            nc.sync.dma_start(b2x_sb[:], b2x[:])
            lba_sb = cpools.tile([16, 3, 256], f32r)
            nc.sync.dma_start(lba_sb[:], lba[:])
            convb_sb = cpools.tile([128, 2], f32)
            nc.sync.dma_start(convb_sb[:], convb[:])
            wbase_sb = cpools.tile([128, 9, 2, 256], bf16)
            nc.sync.dma_start(wbase_sb[:, 0:2], wbase[:, 0:2])

            # ---- x image buffers (bi=0 now; bi=1 later) ----
            xf = [[None] * 2 for _ in range(NB)]

            def load_x(bi):
                for j in range(2):
                    xx = xpool.tile([128, PITCH * (H + 2)], bf16, tag="ximg")
                    xr = xx[:].rearrange("p (a b) -> p a b", b=PITCH)
                    nc.vector.memset(xr[:, 0:1, :], 0.0)
                    nc.vector.memset(xr[:, H + 1:H + 2, :], 0.0)
                    nc.vector.memset(xr[:, :, 0:1], 0.0)
                    nc.vector.memset(xr[:, :, W + 1:W + 2], 0.0)
                    for r0, r1 in ((0, H // 2), (H // 2, H)):
                        nc.sync.dma_start(xr[:, 1 + r0:1 + r1, 1:W + 1],
                                          x_loc[bi, 128 * j:128 * (j + 1), r0:r1, :])
                    xf[bi][j] = xx

            load_x(0)
            nc.sync.dma_start(wbase_sb[:, 2:9], wbase[:, 2:9])

            # ---- MLP (shared) + AS for both samples ----
            coff_sb = []
            with ExitStack() as actx:
                mlp_sb = actx.enter_context(tc.tile_pool(name="mlpw", bufs=1))
                ps_h = actx.enter_context(tc.tile_pool(name="psh", bufs=1, space="PSUM"))
                ps_c = actx.enter_context(tc.tile_pool(name="psc", bufs=2, space="PSUM"))
                ps_a = actx.enter_context(tc.tile_pool(name="psa", bufs=1, space="PSUM"))

                loraA1_sb = mlp_sb.tile([16, 768], f32, tag="la", bufs=2)
                loraA2_sb = mlp_sb.tile([16, 768], f32, tag="la", bufs=2)
                loraA_sb = [loraA1_sb, loraA2_sb]
                nc.sync.dma_start(loraA_sb[0][:], loraA[0:16, :])
                nc.sync.dma_start(loraA_sb[1][:], loraA[16:32, :])

                haug = mlp_sb.tile([128, 2 * NB], f32)
                for br in range(2):
                    h_ps = ps_h.tile([128, NB], f32, tag="hps")
                    nc.tensor.matmul(h_ps[:], ew1_sb[:, 128 * br:128 * (br + 1)],
                                     wm_sb[:, NB * br:NB * (br + 1)], start=True, stop=True)
                    # leaky relu slope 0.2 == max(0.2*x, x)
                    h_sb = mlp_sb.tile([128, NB], f32, tag="hsb", bufs=2)
                    nc.scalar.activation(h_sb[:], h_ps[:], AF.Copy)
                    nc.vector.scalar_tensor_tensor(
                        haug[:, NB * br:NB * (br + 1)], h_sb[:], 0.2, h_sb[:],
                        mybir.AluOpType.mult, mybir.AluOpType.max)
                # stage 2 -> coff[br] (16, 16, NB) [q, r, bi]
                b2x_r = b2x_sb[:, :].rearrange("q (br r b) -> q br r b", br=2, b=NB)
                for br in range(2):
                    c_ps = ps_c.tile([16, 16, NB], f32, tag="cps")
                    for r in range(16):
                        nc.tensor.matmul(c_ps[:, r, :],
                                         w2t_sb[:, 256 * br + 16 * r:256 * br + 16 * (r + 1)],
                                         haug[:, NB * br:NB * (br + 1)],
                                         start=True, stop=True)
                    csb = cpools.tile([16, 16, NB], f32, tag="coff", bufs=2)
                    nc.vector.tensor_add(csb[:], c_ps[:], b2x_r[:, br])
                    coff_sb.append(csb)
                # AS[bi] = coff1[bi].T @ A1 + coff2[bi].T @ A2  -> ASE center
                for bi in range(NB):
                    a_ps = ps_a.tile([16, 768], f32, tag="aps")
                    for c0, c1 in ((0, 512), (512, 768)):
                        nc.tensor.matmul(a_ps[:, c0:c1], coff_sb[0][:, :, bi],
                                         loraA_sb[0][:, c0:c1], start=True, stop=False)
                        nc.tensor.matmul(a_ps[:, c0:c1], coff_sb[1][:, :, bi],
                                         loraA_sb[1][:, c0:c1], start=False, stop=True)
                    nc.vector.tensor_copy(ASE[bi][:, 768:1536], a_ps[:])

            w18 = [[[None] * 2 for _ in range(9)] for _ in range(NB)]

            def emit_wgen(bi):
                # W18[bi][t][j] = (AS expansion) @ lba + wbase, in bf16
                ase_r = ASE[bi][:].rearrange("p (c n) -> p c n", n=9)
                for t in range(9):
                    for j in range(2):
                        wg = ps_w.tile([128, 256], f32, tag="wg")
                        for idx, a in enumerate((j, j + 1)):
                            base = 768 + 1152 * j + t - 768 * a
                            c0, n0 = divmod(base, 9)
                            nc.tensor.matmul(wg[:], ase_r[:, c0:c0 + 128, n0],
                                             lba_sb[:, a, :],
                                             start=(idx == 0), stop=(idx == 1))
                        wt = w18pool.tile([128, 256], bf16, tag="w18")
                        nc.vector.tensor_add(wt[:], wg[:], wbase_sb[:, t, j])
                        w18[bi][t][j] = wt

            emit_wgen(0)

            # ---- the conv ----
            with ExitStack() as bctx:
                cps = bctx.enter_context(tc.tile_pool(name="cps", bufs=6, space="PSUM"))

                def conv_pass(bi, oc):
                    for k in range(NTILE):
                        nr = 3 if k < NTILE - 1 else 2
                        n = PITCH * (nr - 1) + W
                        ps = cps.tile([128, 512], f32, tag="cps")
                        w = 0
                        for kh in range(3):
                            for kw in range(3):
                                t = 3 * kh + kw
                                off = PITCH * (3 * k + kh) + kw
                                for j in range(2):
                                    nc.tensor.matmul(
                                        ps[:, 0:n],
                                        w18[bi][t][j][:, 128 * oc:128 * (oc + 1)],
                                        xf[bi][j][:, off:off + n],
                                        start=(w == 0), stop=(w == 17))
                                    w += 1
                        st = stg.tile([128, 3, 128], f32, tag="stg")
                        pv = ps[:, 0:PITCH * nr].rearrange("p (a b) -> p a b", b=PITCH)
                        nc.vector.tensor_scalar_add(st[:, 0:nr, :], pv[:, :, 0:W],
                                                    convb_sb[:, oc:oc + 1])
                        nc.sync.dma_start(
                            out[bi, 128 * oc:128 * (oc + 1), 3 * k:3 * k + nr, :],
                            st[:, 0:nr, :])

                conv_pass(0, 0)
                emit_wgen(1)
                load_x(1)
                conv_pass(0, 1)
                conv_pass(1, 0)
                conv_pass(1, 1)
    nc.finalize()
    return nc


def _host_prep(inputs):
    """Prepare replicated / per-core numpy input maps."""
    x = np.asarray(inputs["x"], dtype=np.float32)
    wms = np.asarray(inputs["wms"], dtype=np.float32)
    conv_w = np.asarray(inputs["conv_w"], dtype=np.float32)
    conv_b = np.asarray(inputs["conv_b"], dtype=np.float32)
    e_w1 = [np.asarray(inputs["e1_w1"], np.float32), np.asarray(inputs["e2_w1"], np.float32)]
    e_b1 = [np.asarray(inputs["e1_b1"], np.float32), np.asarray(inputs["e2_b1"], np.float32)]
    e_w2 = [np.asarray(inputs["e1_w2"], np.float32), np.asarray(inputs["e2_w2"], np.float32)]
    e_b2 = [np.asarray(inputs["e1_b2"], np.float32), np.asarray(inputs["e2_b2"], np.float32)]
    lora_A = [np.asarray(inputs["lora_A1"], np.float32), np.asarray(inputs["lora_A2"], np.float32)]
    lora_B = np.asarray(inputs["lora_B"], np.float32)

    ew1 = np.zeros((33, 256), np.float32)
    for br in range(2):
        ew1[:32, 128 * br:128 * (br + 1)] = e_w1[br].T
        ew1[32, 128 * br:128 * (br + 1)] = e_b1[br]
    w2t = np.concatenate([e_w2[0].T, e_w2[1].T], axis=1).astype(np.float32)
    b2x = np.zeros((16, 2, 16, NB), np.float32)
    for br in range(2):
        b2x[:, br, :, :] = e_b2[br].reshape(16, 16).T[:, :, None]
    b2x = np.ascontiguousarray(b2x.reshape(16, 2 * 16 * NB))
    loraA = np.concatenate([lora_A[0], lora_A[1]], 0).astype(np.float32)
    # lba[r, a, cout] = lora_B[3*cout + a, r]
    lba = np.ascontiguousarray(lora_B.reshape(256, 3, 16).transpose(2, 1, 0))
    # wbase[p, t, j, cout] = conv_w[cout, 128j+p, t//3, t%3]
    wbase = np.ascontiguousarray(
        conv_w.transpose(2, 3, 1, 0).reshape(9, 2, 128, 256).transpose(2, 0, 1, 3)
    ).astype(ml_dtypes.bfloat16)
    convb = np.ascontiguousarray(conv_b.reshape(2, 128).T)

    xb = x.astype(ml_dtypes.bfloat16)
    in_maps = []
    for core in range(NCORES):
        b0 = core * NB
        wmc = np.ones((33, 2 * NB), np.float32)
        for br in range(2):
            for bi in range(NB):
                wmc[:32, NB * br + bi] = wms[br, b0 + bi]
        in_maps.append({
            "x_loc": np.ascontiguousarray(xb[b0:b0 + NB]),
            "wm": wmc, "ew1": ew1, "w2t": w2t, "b2x": b2x,
            "loraA": loraA, "lba": lba, "wbase": wbase, "convb": convb,
        })
    return in_maps


_NC = None


def kernel(**inputs) -> np.ndarray:
    global _NC
    if _NC is None:
        _NC = _build_nc()
    in_maps = _host_prep(inputs)
    res = run_bass_kernel_spmd(_NC, in_maps, core_ids=list(range(NCORES)))
    return np.concatenate([res.results[c]["out"] for c in range(NCORES)], axis=0)
